# revision 22
# baseline (speedup 1.0000x reference)
"""LSTM decoder (constant input per step, ragged lengths) on 8 TRN2 cores.

Math (per batch element b, for t < seq_len[b]):
    x_proj = Z @ W_ih.T + b_ih + b_hh            (constant over time)
    gates_t = x_proj + h_t @ W_hh.T
    i,f,g,o = split(gates_t); c = sig(f)*c + sig(i)*tanh(g); h = sig(o)*tanh(c)
    ys[b, t] = h_{t+1}

The recurrence is chaotic: 11-bit (fp32r-native) rounding of h or W diverges
past the 2e-2 gate by t~500, so products must be fp32-exact -> Veltkamp 12|12
hi/lo splits of both operands, fp32r streams (fp32r = fp32 stored, rounded to
12 significant bits at the PE input; 12-bit pieces stream exactly).

Device strategy (data-parallel over batch, 16 sequences per core):
  * Streaming matmul: moving = W_hh.T column blocks [128, 512] fp32r
    (1 cycle/row), stationary = h.T chunks hi/lo M-stacked so one moving
    pass of w_hi yields hi*wh AND lo*wh; the w_lo pass adds the rest.
    2 moving passes per (k-chunk, n-block) = 32 MMs/step, the PE floor.
  * Evacuation without Act copies: DVE adds read PSUM rows directly.
    Groups n3/n2 keep a PE xp-matmul (start-of-group stall filler) and
    merge with ONE DVE add (ps_hi + ps_lo); groups n0/n1 fold x_proj on
    DVE (t1 = ps_hi + xp; ga = t1 + ps_lo) to shed PE work.
  * Gate order i|f|o|g per 128-hidden block: one sigmoid over 384 cols +
    one tanh over 128, reading SBUF ga.
  * c-chain: early groups on GpSimd (idle engine), late groups on DVE to
    shorten the step tail; tanh on Act; h-split hi-copy on Act, lo-sub DVE.
  * Four per-chunk T32 transposes placed individually in the PE stream so
    each next-step k-pass finds its stationary split just in time.
"""

import numpy as np

import concourse.bass as bass
import concourse.tile as tile
from concourse import bacc, mybir
from concourse.bass_utils import run_bass_kernel_spmd

B, F, H, TMAX = 128, 128, 512, 512
N_CORES = 8
BL = B // N_CORES          # local batch = 16
NB = 4                     # hidden blocks of 128 (= N chunks and K chunks)
T_STEPS = TMAX - 1         # seq_len < 512, so at most 511 steps matter
SPLIT_C = float(2.0 ** 12 + 1)

FP32 = mybir.dt.float32
FP32R = mybir.dt.float32r
AF = mybir.ActivationFunctionType

# groups processed in this order each step; SLOT[n] = h1 partition slot.
# Late chunks (1, 2) get bases 0/32 so they can be single-transposed (base
# partition must be 0/32/64); early chunks (3, 0) pair-transpose at base 64.
# sP column layout is SLOT-major: chunk k at cols 48*SLOT[k].
ORDER = (3, 0, 1, 2)
SLOT = {1: 0, 2: 1, 3: 2, 0: 3}
# groups whose x_proj folds into the DVE evac adds (no PE xp-matmul)
XP_FOLD = (3, 0)


def _split12(x):
    x = x.astype(np.float32)
    v = (x * np.float32(SPLIT_C)).astype(np.float32)
    hi = (v - (v - x).astype(np.float32)).astype(np.float32)
    lo = (x - hi).astype(np.float32)
    return hi, lo


def build_lstm_nc(t_steps: int = T_STEPS):
    """Build + compile the per-core Bass program (SPMD: same NEFF, 8 cores)."""
    nc = bacc.Bacc("TRN2", target_bir_lowering=False, debug=False)

    wrh_d = nc.dram_tensor("wrh", [128, NB * 2048], FP32R, kind="ExternalInput")
    wrl_d = nc.dram_tensor("wrl", [128, NB * 2048], FP32R, kind="ExternalInput")
    wih_d = nc.dram_tensor("wih", [128, 2048], FP32R, kind="ExternalInput")
    wil_d = nc.dram_tensor("wil", [128, 2048], FP32R, kind="ExternalInput")
    z_d = nc.dram_tensor("z", [128, 48], FP32R, kind="ExternalInput")  # [hi|0|lo]
    bias_d = nc.dram_tensor("bias", [2, 2048], FP32R, kind="ExternalInput")  # hi/lo rows
    sb_d = nc.dram_tensor("sb", [2, 48], FP32R, kind="ExternalInput")  # ones selector
    sx_d = nc.dram_tensor("sx", [48, 48], FP32R, kind="ExternalInput")  # xp selector
    eye_d = nc.dram_tensor("eye", [128, 128], FP32, kind="ExternalInput")
    # ys stored hid-block-major: [t, s, b, q] with slot s = chunk ORDER[s]
    ys_d = nc.dram_tensor("ys", [t_steps, NB, BL, 128], FP32, kind="ExternalOutput")

    with tile.TileContext(nc) as tc:
        with (
            tc.tile_pool(name="const", bufs=1) as constp,
            tc.tile_pool(name="state", bufs=1) as statep,
            tc.tile_pool(name="work", bufs=4) as workp,
            tc.tile_pool(name="hout", bufs=4) as houtp,
            tc.tile_pool(name="ps", bufs=4, space="PSUM") as psp,
            tc.tile_pool(name="pst", bufs=2, space="PSUM") as pstp,
        ):
            # --- constants ---
            wrh = constp.tile([128, NB * 2048], FP32R)
            nc.sync.dma_start(wrh[:], wrh_d.ap())
            wrl = constp.tile([128, NB * 2048], FP32R)
            nc.sync.dma_start(wrl[:], wrl_d.ap())
            wih = constp.tile([128, 2048], FP32R)
            nc.sync.dma_start(wih[:], wih_d.ap())
            wil = constp.tile([128, 2048], FP32R)
            nc.sync.dma_start(wil[:], wil_d.ap())
            s_z = constp.tile([128, 48], FP32R)
            nc.sync.dma_start(s_z[:], z_d.ap())
            bias2 = constp.tile([2, 2048], FP32R)
            nc.sync.dma_start(bias2[:2, :], bias_d.ap())
            s_b = constp.tile([2, 48], FP32R)
            nc.sync.dma_start(s_b[:2, :], sb_d.ap())
            s_x = constp.tile([48, 48], FP32R)
            nc.sync.dma_start(s_x[:48, :], sx_d.ap())
            eye = constp.tile([128, 128], FP32)
            nc.sync.dma_start(eye[:], eye_d.ap())

            # --- state ---
            c1 = statep.tile([BL, H], FP32)          # cell, layout 1
            nc.vector.memset(c1[:BL, :], 0.0)
            # stationary state, double buffered: sP[p] [128, 192] fp32r,
            # chunk k at cols [48k, 48k+48): hi at +0:16, pad, lo at +32:48
            sP = [
                statep.tile([128, 192], FP32R, tag=f"sP{j}", name=f"sP{j}")
                for j in range(2)
            ]
            zf = statep.tile([128, 192], FP32)
            nc.vector.memset(zf[:], 0.0)
            nc.vector.tensor_copy(sP[0][:], zf[:])
            nc.vector.tensor_copy(sP[1][:], zf[:])

            # --- one-time x_proj + bias ---
            # xpb: full-fp32 x_proj for the DVE-fold groups
            xpb = statep.tile([BL, 2048], FP32)
            # xp_mov: [hi;lo] K-stacked moving tile for the PE-xp groups
            zf2 = statep.tile([48, 2048], FP32)
            nc.vector.memset(zf2[:48, :], 0.0)
            xp_mov = constp.tile([48, 2048], FP32R)
            nc.vector.tensor_copy(xp_mov[:48, :], zf2[:48, :])

            def emit_zbias(ps, n, stop=False):
                """x_proj+bias: two moving passes of W_ih (z hi/lo M-stacked)
                + one K=2 ones-matmul adding bias hi+lo."""
                wi_h = wih[:, n * 512 : (n + 1) * 512]
                wi_l = wil[:, n * 512 : (n + 1) * 512]
                nc.tensor.matmul(ps[:48, :], s_z[:, 0:48], wi_h,
                                 start=True, stop=False)
                nc.tensor.matmul(ps[:48, :], s_z[:, 0:48], wi_l,
                                 start=False, stop=False)
                nc.tensor.matmul(ps[:48, :], s_b[0:2, 0:48],
                                 bias2[0:2, n * 512 : (n + 1) * 512],
                                 start=False, stop=stop)

            for n in range(NB):
                psx = psp.tile([48, 512], FP32, tag="gates", name="psg")
                emit_zbias(psx, n, stop=True)
                xtmp = workp.tile([BL, 512], FP32, tag="lo", name="lo_sb")
                nc.scalar.activation(xtmp[:BL, :], psx[32:48, :], AF.Copy)
                xpn = xpb[:BL, n * 512 : (n + 1) * 512]
                nc.vector.tensor_add(xpn, psx[0:BL, :], xtmp[:BL, :])
                # hi/lo split for the PE-xp moving tile (rounding happens on
                # the fp32->fp32r write)
                nc.vector.tensor_copy(
                    xp_mov[0:16, n * 512 : (n + 1) * 512], xpn
                )
                nc.vector.tensor_sub(
                    xp_mov[32:48, n * 512 : (n + 1) * 512], xpn,
                    xp_mov[0:16, n * 512 : (n + 1) * 512],
                )

            # --- per-step emitters ---
            def emit_xp_mm(ps, n):
                nc.tensor.matmul(ps[:48, :], s_x[0:48, 0:48],
                                 xp_mov[0:48, n * 512 : (n + 1) * 512],
                                 start=True, stop=False)

            def emit_mm_pairs(ps, n, s_p, ks, start, stop):
                for ki, k in enumerate(ks):
                    w_h = wrh[:, k * 2048 + n * 512 : k * 2048 + (n + 1) * 512]
                    w_l = wrl[:, k * 2048 + n * 512 : k * 2048 + (n + 1) * 512]
                    s_k = s_p[:, SLOT[k] * 48 : SLOT[k] * 48 + 48]
                    nc.tensor.matmul(ps[:48, :], s_k, w_h,
                                     start=(start and ki == 0), stop=False)
                    nc.tensor.matmul(ps[:48, :], s_k, w_l, start=False,
                                     stop=(stop and ki == len(ks) - 1))

            def emit_evac(n, ps):
                """PSUM -> activated gates: Act evacuates the lo half (DVE
                TensorTensor requires partition-aligned operands), DVE adds
                merge. XP_FOLD groups add x_proj on DVE (their PE xp-matmul
                is omitted); all operands stay at partition base 0."""
                lo_sb = workp.tile([BL, 512], FP32, tag="lo", name="lo_sb")
                nc.scalar.activation(lo_sb[:BL, :], ps[32:48, :], AF.Copy)
                ga = workp.tile([BL, 512], FP32, tag="ga", name="ga")
                nc.vector.tensor_add(ga[:BL, :], ps[0:BL, :], lo_sb[:BL, :])
                act = workp.tile([BL, 512], FP32, tag="act", name="act")
                nc.scalar.activation(act[:BL, 0:384], ga[:BL, 0:384], AF.Sigmoid)
                nc.scalar.activation(act[:BL, 384:512], ga[:BL, 384:512], AF.Tanh)
                return act

            def emit_cchain(n, act, h1):
                i_s = act[:BL, 0:128]
                f_s = act[:BL, 128:256]
                o_s = act[:BL, 256:384]
                g_s = act[:BL, 384:512]
                cn = c1[:BL, n * 128 : (n + 1) * 128]
                # i*g runs on GpSimd in parallel with DVE's f*c; the rest of
                # the chain stays on DVE (GpSimd ops are ~2.5x slower and
                # would delay the transposes that gate the next step).
                t1 = workp.tile([BL, 128], FP32, tag="t1", name="t1")
                nc.gpsimd.tensor_mul(t1[:BL, :], i_s, g_s)
                nc.vector.tensor_mul(cn, f_s, cn)
                nc.vector.tensor_add(cn, cn, t1[:BL, :])
                tct = workp.tile([BL, 128], FP32, tag="tct", name="tct")
                nc.scalar.activation(tct[:BL, :], cn, AF.Tanh)
                hn = h1[32 * SLOT[n] : 32 * SLOT[n] + BL, :]
                nc.vector.tensor_mul(hn, o_s, tct[:BL, :])

            def emit_t32(n, h1, s_n):
                """Transpose one late chunk's h slot (base 0/32) + hi/lo
                split into s_n. hi-copy on Act (rounds fp32->fp32r),
                lo-sub on DVE."""
                base = 32 * SLOT[n]
                psT = pstp.tile([128, 32], FP32, tag="psT", name="psT")
                nc.tensor.transpose(
                    psT[:, 0:32], h1[base : base + 32, :],
                    eye[base : base + 32, base : base + 32],
                )
                col = 48 * SLOT[n]
                hi = s_n[:, col : col + 16]
                lo = s_n[:, col + 32 : col + 48]
                nc.vector.tensor_copy(hi, psT[:, 0:BL])
                nc.vector.tensor_sub(lo, psT[:, 0:BL], hi)

            def emit_t64_pair(h1, s_n):
                """Pair-transpose slots 2,3 (chunks 3, 0) at base 64, then
                split both with 2-level free APs (slot-major sP layout)."""
                psT = pstp.tile([128, 64], FP32, tag="psT64", name="psT64")
                nc.tensor.transpose(
                    psT[:, 0:64], h1[64:128, :], eye[64:128, 64:128]
                )
                dst = s_n[:, 96:192].rearrange("p (k c) -> p k c", c=48)
                src = psT[:, 0:64].rearrange("p (k c) -> p k c", c=32)[:, :, 0:16]
                nc.vector.tensor_copy(dst[:, :, 0:16], src)
                nc.vector.tensor_sub(dst[:, :, 32:48], src, dst[:, :, 0:16])

            # --- recurrence ---
            n3, n0, n1, n2 = ORDER  # 3, 0, 1, 2
            prev_h1 = None
            for t in range(t_steps):
                s_p = sP[t % 2]
                s_n = sP[(t + 1) % 2]
                h1 = houtp.tile([128, 128], FP32, tag="h1", name="h1")

                ps = {n: psp.tile([48, 512], FP32, tag="gates", name="psg")
                      for n in ORDER}
                acts = {}

                # G3: xp MM + k3 pair, then prev-step chunk1 transpose,
                # k0/k1 pairs, prev-step chunk2 transpose, k2 pair (stop).
                emit_xp_mm(ps[n3], n3)
                emit_mm_pairs(ps[n3], n3, s_p, (3,), start=False, stop=False)
                if prev_h1 is not None:
                    emit_t32(n1, prev_h1, s_p)
                emit_mm_pairs(ps[n3], n3, s_p, (0, 1), start=False, stop=False)
                if prev_h1 is not None:
                    emit_t32(n2, prev_h1, s_p)
                emit_mm_pairs(ps[n3], n3, s_p, (2,), start=False, stop=True)
                acts[n3] = emit_evac(n3, ps[n3])

                # G0 (cchain n3 emitted before this group's evac so its DVE
                # ops precede the merge in the FIFO)
                emit_xp_mm(ps[n0], n0)
                emit_mm_pairs(ps[n0], n0, s_p, ORDER, start=False, stop=True)
                emit_cchain(n3, acts[n3], h1)
                acts[n0] = emit_evac(n0, ps[n0])

                # G1
                emit_xp_mm(ps[n1], n1)
                emit_mm_pairs(ps[n1], n1, s_p, ORDER, start=False, stop=True)
                emit_cchain(n0, acts[n0], h1)
                acts[n1] = emit_evac(n1, ps[n1])

                # G2: xp MM + 8 W passes; pair-transpose of the two early
                # chunks (3, 0) lands mid-stream right after cchain(n0);
                # cchain(n1) follows the pair in the DVE stream so the split
                # is not delayed behind it.
                emit_xp_mm(ps[n2], n2)
                emit_cchain(n1, acts[n1], h1)
                emit_mm_pairs(ps[n2], n2, s_p, ORDER[:2], start=False, stop=False)
                if t < t_steps - 1:
                    emit_t64_pair(h1, s_n)
                emit_mm_pairs(ps[n2], n2, s_p, ORDER[2:], start=False, stop=True)
                acts[n2] = emit_evac(n2, ps[n2])
                emit_cchain(n2, acts[n2], h1)

                # ys DMA per slot: h1[32s:32s+16, :] -> ys[t, s]
                for s in range(NB):
                    nc.sync.dma_start(
                        ys_d.ap()[t, s], h1[32 * s : 32 * s + BL, :]
                    )
                prev_h1 = h1

    nc.compile()
    return nc


def _prep_host_inputs(Z, seq_len, W_ih, W_hh, b_ih, b_hh):
    """Per-core in_maps with device-native layouts."""
    WT = np.ascontiguousarray(W_hh.astype(np.float32).T)      # [H, 4H] (hid_in, gate)
    WIT = np.ascontiguousarray(W_ih.astype(np.float32).T)     # [F, 4H]
    bias = (b_ih.astype(np.float32) + b_hh.astype(np.float32))

    # column reorder: col = n*512 + r*128 + q  <->  gate index G(r)*H + 128n + q
    # with in-chunk gate order G = (i, f, o, g) so sigmoid covers cols 0:384.
    GMAP = np.array([0, 1, 3, 2])
    n_i = np.arange(2048)
    nn, rem = np.divmod(n_i, 512)
    rr, qq = np.divmod(rem, 128)
    colmap = GMAP[rr] * H + 128 * nn + qq                     # [2048]

    wr_np = np.empty((128, NB * 2048), dtype=np.float32)
    for k in range(NB):
        wr_np[:, k * 2048 : (k + 1) * 2048] = WT[k * 128 : (k + 1) * 128, colmap]
    wrh_np, wrl_np = _split12(wr_np)
    wih_np, wil_np = _split12(np.ascontiguousarray(WIT[:, colmap]))
    b_hi, b_lo = _split12(bias[colmap])
    bias_np = np.stack([b_hi, b_lo])                          # [2, 2048]
    sb_np = np.zeros((2, 48), dtype=np.float32)
    sb_np[:, 0:16] = 1.0
    sx_np = np.zeros((48, 48), dtype=np.float32)
    sx_np[0:16, 0:16] = np.eye(16)
    sx_np[32:48, 0:16] = np.eye(16)
    eye_np = np.eye(128, dtype=np.float32)

    in_maps = []
    for c in range(N_CORES):
        zc = np.ascontiguousarray(Z[c * BL : (c + 1) * BL].astype(np.float32).T)
        z_hi, z_lo = _split12(zc)
        z_np = np.zeros((128, 48), dtype=np.float32)
        z_np[:, 0:16] = z_hi
        z_np[:, 32:48] = z_lo
        in_maps.append(
            {"wrh": wrh_np, "wrl": wrl_np, "wih": wih_np, "wil": wil_np,
             "z": z_np, "bias": bias_np, "eye": eye_np, "sb": sb_np,
             "sx": sx_np}
        )
    return in_maps


_NC_CACHE = {}


def get_nc(t_steps: int = T_STEPS):
    if t_steps not in _NC_CACHE:
        _NC_CACHE[t_steps] = build_lstm_nc(t_steps)
    return _NC_CACHE[t_steps]


def kernel(Z, seq_len, W_ih, W_hh, b_ih, b_hh, _trace=False, _tmpdir=None):
    nc = get_nc()
    in_maps = _prep_host_inputs(Z, seq_len, W_ih, W_hh, b_ih, b_hh)
    res = run_bass_kernel_spmd(
        nc, in_maps, core_ids=list(range(N_CORES)), trace=_trace, tmpdir=_tmpdir
    )
    kernel.last_result = res

    out = np.zeros((B, TMAX, H), dtype=np.float32)
    for c in range(N_CORES):
        ys = res.results[c]["ys"]  # [T_STEPS, slot, BL, 128]; chunk n at slot SLOT[n]
        for n in range(NB):
            out[c * BL : (c + 1) * BL, :T_STEPS, n * 128 : (n + 1) * 128] = (
                ys[:, SLOT[n]].transpose(1, 0, 2)
            )
    mask = np.arange(TMAX, dtype=np.int64)[None, :] < seq_len.astype(np.int64)[:, None]
    out *= mask[:, :, None].astype(np.float32)
    return out
